# revision 4
# baseline (speedup 1.0000x reference)
"""Multi-head attention (B=4, S=2048, H=512, NH=8) on 8 trn2 NeuronCores.

Sharding: core c handles batch b = c//2, query half = c%2 (1024 queries),
full 2048-key attention for all 8 heads, plus its slice of the QKV/out
projections.  No cross-core reduction is needed: each core produces final
output rows [b, half*1024:(half+1)*1024, :].

Per-core layout strategy (everything transposed, features on partitions):
  xT     [512, 2048]  input transposed (rotated so first 1024 cols = queries)
  QT     [512, 1024]  q features x queries          (bf16)
  KT     [512, 2048]  k features x keys             (bf16)
  V      [2048, 8*65] keys x (per-head 64 vals + ones col)  (bf16)
  logitsT[k, q] tiles via row-tiled (2 heads concurrent) K=64 matmuls
  pT = exp(scale*logitsT + maskbias_k): mask bias is per-partition (k on
       partitions), no max subtraction (logits bounded ~ +-10 after scale)
  attn.V: V-block stationary (M=65: 64 vals + ones col -> softmax sums ride
       along as PSUM row 64), pT moving
  normalize: vals / sums via reciprocal + DMA partition-broadcast + DVE mul
  o-proj: valsT [f, s] slices are the stationary operand directly -> out [s, 512]
"""

import numpy as np
import ml_dtypes

B, S, H, NH, DH = 4, 2048, 512, 8, 64
SQ = S // 2          # queries per core
N_CORES = 8
SCALE = 1.0 / 8.0    # 1/sqrt(64)
NEG_BIAS = -1e9      # pre-scale additive mask bias for padded keys

KT_TILES = S // 128       # 16 key tiles of 128
QW = 512                  # query window (psum free size)
N_QW = SQ // QW           # 2
FT = H // 128             # 4 feature tiles
ST = S // 128             # 16 s-tiles for V
SOT = SQ // 128           # 8 s-tiles for output

_CACHE = {}


def _build_nc():
    import concourse.bass as bass
    import concourse.mybir as mybir
    import concourse.tile as tile
    from concourse import bacc

    dt = mybir.dt
    AF = mybir.ActivationFunctionType

    nc = bacc.Bacc("TRN2", target_bir_lowering=False)

    # ---- DRAM I/O (per-core) ----
    d_xT = nc.dram_tensor("xT", [FT, 128, S], dt.bfloat16, kind="ExternalInput")
    d_wq = nc.dram_tensor("wq", [FT, 128, H], dt.bfloat16, kind="ExternalInput")
    d_wk = nc.dram_tensor("wk", [FT, 128, H], dt.bfloat16, kind="ExternalInput")
    d_wv = nc.dram_tensor("wv", [FT, 128, H], dt.bfloat16, kind="ExternalInput")
    d_wo = nc.dram_tensor("wo", [FT, 128, H], dt.bfloat16, kind="ExternalInput")
    d_bq = nc.dram_tensor("bq", [FT, 128, 1], dt.float32, kind="ExternalInput")
    d_bk = nc.dram_tensor("bk", [FT, 128, 1], dt.float32, kind="ExternalInput")
    d_bv = nc.dram_tensor("bv", [1, H], dt.bfloat16, kind="ExternalInput")
    d_bo = nc.dram_tensor("bo", [1, H], dt.bfloat16, kind="ExternalInput")
    d_mb = nc.dram_tensor("mb", [128, KT_TILES], dt.float32, kind="ExternalInput")
    d_out = nc.dram_tensor("out", [SQ, H], dt.float32, kind="ExternalOutput")

    with tile.TileContext(nc) as tc:
        with (
            tc.tile_pool(name="persist", bufs=1) as persist,
            tc.tile_pool(name="pt", bufs=20) as ptp,
            tc.tile_pool(name="work", bufs=4) as work,
            tc.tile_pool(name="outp", bufs=4) as outp,
            tc.tile_pool(name="pp", bufs=2, space="PSUM") as pp,
            tc.tile_pool(name="lgp", bufs=2, space="PSUM") as lgp,
            tc.tile_pool(name="avp", bufs=2, space="PSUM") as avp,
        ):
            # ---- load persistent tensors ----
            xT = [persist.tile([128, S], dt.bfloat16, tag=f"xT{j}", name=f"xT{j}") for j in range(FT)]
            wq = [persist.tile([128, H], dt.bfloat16, tag=f"wq{j}", name=f"wq{j}") for j in range(FT)]
            wk = [persist.tile([128, H], dt.bfloat16, tag=f"wk{j}", name=f"wk{j}") for j in range(FT)]
            wv = [persist.tile([128, H], dt.bfloat16, tag=f"wv{j}", name=f"wv{j}") for j in range(FT)]
            wo = [persist.tile([128, H], dt.bfloat16, tag=f"wo{j}", name=f"wo{j}") for j in range(FT)]
            bq = [persist.tile([128, 1], dt.float32, tag=f"bq{j}", name=f"bq{j}") for j in range(FT)]
            bk = [persist.tile([128, 1], dt.float32, tag=f"bk{j}", name=f"bk{j}") for j in range(FT)]
            bv = persist.tile([1, H], dt.bfloat16, tag="bv", name="bv")
            bo = persist.tile([1, H], dt.bfloat16, tag="bo", name="bo")
            mb = persist.tile([128, KT_TILES], dt.float32, tag="mb", name="mb")
            ones = persist.tile([1, 128], dt.bfloat16, tag="ones", name="ones")

            for j in range(FT):
                nc.sync.dma_start(out=xT[j], in_=d_xT[j])
                nc.sync.dma_start(out=wq[j], in_=d_wq[j])
                nc.sync.dma_start(out=wk[j], in_=d_wk[j])
                nc.sync.dma_start(out=wv[j], in_=d_wv[j])
                nc.sync.dma_start(out=wo[j], in_=d_wo[j])
                nc.sync.dma_start(out=bq[j], in_=d_bq[j])
                nc.sync.dma_start(out=bk[j], in_=d_bk[j])
            nc.sync.dma_start(out=bv, in_=d_bv[:, :])
            nc.sync.dma_start(out=bo, in_=d_bo[:, :])
            nc.sync.dma_start(out=mb, in_=d_mb[:, :])
            nc.vector.memset(ones, 1.0)

            QT = [persist.tile([128, SQ], dt.bfloat16, tag=f"QT{f}", name=f"QT{f}") for f in range(FT)]
            KT = [persist.tile([128, S], dt.bfloat16, tag=f"KT{f}", name=f"KT{f}") for f in range(FT)]
            Vt = [persist.tile([128, NH, DH + 1], dt.bfloat16, tag=f"V{s}", name=f"V{s}")
                  for s in range(ST)]
            valsT = [persist.tile([128, SQ], dt.bfloat16, tag=f"vT{f}", name=f"vT{f}")
                     for f in range(FT)]

            # ---- QT / KT projections for feature tile 0 first (unblocks attn) ----
            def proj_qkt(f):
                for w in range(N_QW):
                    ps = pp.tile([128, QW], dt.float32, tag="pp", name="pp")
                    for j in range(FT):
                        nc.tensor.matmul(
                            ps, lhsT=wq[j][:, f * 128:(f + 1) * 128],
                            rhs=xT[j][:, w * QW:(w + 1) * QW],
                            start=(j == 0), stop=(j == FT - 1))
                    nc.vector.tensor_scalar_add(
                        QT[f][:, w * QW:(w + 1) * QW], ps, bq[f])
                for w in range(S // QW):
                    ps = pp.tile([128, QW], dt.float32, tag="pp", name="pp")
                    for j in range(FT):
                        nc.tensor.matmul(
                            ps, lhsT=wk[j][:, f * 128:(f + 1) * 128],
                            rhs=xT[j][:, w * QW:(w + 1) * QW],
                            start=(j == 0), stop=(j == FT - 1))
                    nc.vector.tensor_scalar_add(
                        KT[f][:, w * QW:(w + 1) * QW], ps, bk[f])

            proj_qkt(0)

            # ---- V projection (keys x values, all heads), ones column ----
            for s in range(ST):
                ps = pp.tile([128, H], dt.float32, tag="pp", name="pp")
                for j in range(FT):
                    nc.tensor.matmul(
                        ps, lhsT=xT[j][:, s * 128:(s + 1) * 128], rhs=wv[j],
                        start=(j == 0), stop=False)
                nc.tensor.matmul(ps, lhsT=ones, rhs=bv, start=False, stop=True)
                nc.vector.tensor_copy(
                    Vt[s][:, :, 0:DH],
                    ps.rearrange("p (h d) -> p h d", h=NH))
                nc.vector.memset(Vt[s][:, :, DH:DH + 1], 1.0)

            for f in range(1, FT):
                proj_qkt(f)

            # ---- attention: head pairs (features of tile hp = heads 2hp, 2hp+1) ----
            for hp in range(FT):
                for w in range(N_QW):
                    qs = slice(w * QW, (w + 1) * QW)
                    pts = []
                    for kt in range(KT_TILES):
                        lg = lgp.tile([128, 2 * QW], dt.float32, tag="lg", name="lg")
                        nc.tensor.matmul(
                            lg[:, 0:QW],
                            lhsT=KT[hp][0:64, kt * 128:(kt + 1) * 128],
                            rhs=QT[hp][0:64, qs],
                            start=True, stop=True, tile_position=(0, 0))
                        nc.tensor.matmul(
                            lg[:, QW:2 * QW],
                            lhsT=KT[hp][64:128, kt * 128:(kt + 1) * 128],
                            rhs=QT[hp][64:128, qs],
                            start=True, stop=True, tile_position=(64, 0))
                        pt = ptp.tile([128, 2 * QW], dt.bfloat16, tag="pt", name="pt")
                        nc.scalar.activation(
                            pt, lg, AF.Exp,
                            bias=mb[:, kt:kt + 1], scale=SCALE)
                        pts.append(pt)
                    att = [avp.tile([128, QW], dt.float32, tag="av", name="av") for _ in range(2)]
                    for i in range(2):
                        for kt in range(KT_TILES):
                            nc.tensor.matmul(
                                att[i][0:DH + 1, :],
                                lhsT=Vt[kt][:, 2 * hp + i, :],
                                rhs=pts[kt][:, i * QW:(i + 1) * QW],
                                start=(kt == 0), stop=(kt == KT_TILES - 1))
                    for i in range(2):
                        recip = work.tile([1, QW], dt.float32, tag="recip", name="recip")
                        nc.vector.reciprocal(recip, att[i][DH:DH + 1, :])
                        bc = work.tile([64, QW], dt.float32, tag="bc", name="bc")
                        nc.gpsimd.partition_broadcast(bc[:, :], recip[:, :])
                        nc.vector.tensor_mul(
                            valsT[hp][i * 64:(i + 1) * 64, qs],
                            att[i][0:DH, :], bc)

            # ---- output projection ----
            for s in range(SOT):
                ps = pp.tile([128, H], dt.float32, tag="pp", name="pp")
                for f in range(FT):
                    nc.tensor.matmul(
                        ps, lhsT=valsT[f][:, s * 128:(s + 1) * 128], rhs=wo[f],
                        start=(f == 0), stop=False)
                nc.tensor.matmul(ps, lhsT=ones, rhs=bo, start=False, stop=True)
                ot = outp.tile([128, H], dt.float32, tag="ot", name="ot")
                nc.vector.tensor_copy(ot, ps)
                nc.sync.dma_start(out=d_out[s * 128:(s + 1) * 128, :], in_=ot)

    nc.compile()
    return nc


def _get_nc():
    if "nc" not in _CACHE:
        _CACHE["nc"] = _build_nc()
    return _CACHE["nc"]


def _bf16(a):
    return np.ascontiguousarray(a.astype(ml_dtypes.bfloat16))


def _prep_shared(qkv_w, qkv_b, o_w, o_b):
    """Host-side weight permutation: reference splits qkv per head into
    (q, v, k) chunks of 64 within each head's 192 rows."""
    idx = np.arange(NH)[:, None] * (3 * DH) + np.arange(DH)[None, :]
    q_idx = idx.ravel()            # head-major q rows
    v_idx = (idx + DH).ravel()
    k_idx = (idx + 2 * DH).ravel()

    def wT(rows):
        # [H_in, 512 features] -> [FT, 128, 512]
        return _bf16(qkv_w[rows].T.reshape(H, H)).reshape(FT, 128, H)

    shared = {
        "wq": wT(q_idx), "wk": wT(k_idx), "wv": wT(v_idx),
        "wo": _bf16(o_w.T).reshape(FT, 128, H),
        "bq": np.ascontiguousarray(qkv_b[q_idx].astype(np.float32)).reshape(FT, 128, 1),
        "bk": np.ascontiguousarray(qkv_b[k_idx].astype(np.float32)).reshape(FT, 128, 1),
        "bv": _bf16(qkv_b[v_idx]).reshape(1, H),
        "bo": _bf16(o_b).reshape(1, H),
    }
    return shared


def kernel(x, src_padding_mask, qkv_w, qkv_b, o_w, o_b):
    from concourse.bass_utils import run_bass_kernel_spmd

    x = np.asarray(x, dtype=np.float32)
    mask = np.asarray(src_padding_mask)
    qkv_w = np.asarray(qkv_w, dtype=np.float32)
    qkv_b = np.asarray(qkv_b, dtype=np.float32)
    o_w = np.asarray(o_w, dtype=np.float32)
    o_b = np.asarray(o_b, dtype=np.float32)

    nc = _get_nc()
    shared = _prep_shared(qkv_w, qkv_b, o_w, o_b)

    in_maps = []
    for c in range(N_CORES):
        b, half = c // 2, c % 2
        xr = np.roll(x[b], -SQ * half, axis=0)     # first SQ rows = this core's q
        mr = np.roll(mask[b], -SQ * half)
        mbias = np.where(mr, 0.0, NEG_BIAS).astype(np.float32)
        m = dict(shared)
        m["xT"] = _bf16(xr.T).reshape(FT, 128, S)
        m["mb"] = np.ascontiguousarray(mbias.reshape(KT_TILES, 128).T)
        in_maps.append(m)

    res = run_bass_kernel_spmd(nc, in_maps, core_ids=list(range(N_CORES)))

    out = np.empty((B, S, H), dtype=np.float32)
    for c in range(N_CORES):
        b, half = c // 2, c % 2
        out[b, half * SQ:(half + 1) * SQ] = res.results[c]["out"]
    return out


# revision 8
# speedup vs baseline: 1.0076x; 1.0076x over previous
"""Multi-head attention (B=4, S=2048, H=512, NH=8) on 8 trn2 NeuronCores.

Sharding: core c handles batch b = c//2, query half = c%2 (1024 queries),
full 2048-key attention for all 8 heads, plus its slice of the QKV/out
projections.  No cross-core reduction is needed: each core produces final
output rows [b, half*1024:(half+1)*1024, :].

Per-core layout strategy (everything transposed, features on partitions):
  xT     [512, 2048]  input transposed (rotated so first 1024 cols = queries)
  QT     [512, 1024]  q features x queries          (bf16)
  KT     [512, 2048]  k features x keys             (bf16)
  V      [2048, 8*65] keys x (per-head 64 vals + ones col)  (bf16)
  logitsT[k, q] tiles via row-tiled (2 heads concurrent) K=64 matmuls
  pT = exp(scale*logitsT + maskbias_k): mask bias is per-partition (k on
       partitions), no max subtraction (logits bounded ~ +-10 after scale)
  attn.V: V-block stationary (M=65: 64 vals + ones col -> softmax sums ride
       along as PSUM row 64), pT moving
  normalize: vals / sums via reciprocal + DMA partition-broadcast + DVE mul
  o-proj: valsT [f, s] slices are the stationary operand directly -> out [s, 512]
"""

import numpy as np
import ml_dtypes

B, S, H, NH, DH = 4, 2048, 512, 8, 64
SQ = S // 2          # queries per core
N_CORES = 8
SCALE = 1.0 / 8.0    # 1/sqrt(64)
NEG_BIAS = -1e9      # pre-scale additive mask bias for padded keys

KT_TILES = S // 128       # 16 key tiles of 128
QW = 512                  # query window (psum free size)
N_QW = SQ // QW           # 2
FT = H // 128             # 4 feature tiles
ST = S // 128             # 16 s-tiles for V
SOT = SQ // 128           # 8 s-tiles for output

_CACHE = {}


def _build_nc():
    import concourse.bass as bass
    import concourse.mybir as mybir
    import concourse.tile as tile
    from concourse import bacc

    dt = mybir.dt
    AF = mybir.ActivationFunctionType

    nc = bacc.Bacc("TRN2", target_bir_lowering=False)

    # ---- DRAM I/O (per-core) ----
    d_xT = nc.dram_tensor("xT", [FT, 128, S], dt.bfloat16, kind="ExternalInput")
    d_wq = nc.dram_tensor("wq", [FT, 128, H], dt.bfloat16, kind="ExternalInput")
    d_wk = nc.dram_tensor("wk", [FT, 128, H], dt.bfloat16, kind="ExternalInput")
    d_wv = nc.dram_tensor("wv", [FT, 128, H], dt.bfloat16, kind="ExternalInput")
    d_wo = nc.dram_tensor("wo", [FT, 128, H], dt.bfloat16, kind="ExternalInput")
    d_bq = nc.dram_tensor("bq", [FT, 128, 1], dt.float32, kind="ExternalInput")
    d_bk = nc.dram_tensor("bk", [FT, 128, 1], dt.float32, kind="ExternalInput")
    d_bv = nc.dram_tensor("bv", [1, H], dt.bfloat16, kind="ExternalInput")
    d_bo = nc.dram_tensor("bo", [1, H], dt.bfloat16, kind="ExternalInput")
    d_mb = nc.dram_tensor("mb", [128, KT_TILES], dt.float32, kind="ExternalInput")
    d_out = nc.dram_tensor("out", [SQ, H], dt.float32, kind="ExternalOutput")

    with tile.TileContext(nc) as tc:
        with (
            tc.tile_pool(name="persist", bufs=1) as persist,
            tc.tile_pool(name="pt", bufs=20) as ptp,
            tc.tile_pool(name="work", bufs=4) as work,
            tc.tile_pool(name="outp", bufs=4) as outp,
            tc.tile_pool(name="pp", bufs=2, space="PSUM") as pp,
            tc.tile_pool(name="lgp", bufs=2, space="PSUM") as lgp,
            tc.tile_pool(name="avp", bufs=2, space="PSUM") as avp,
        ):
            # ---- load persistent tensors ----
            xT = [persist.tile([128, S], dt.bfloat16, tag=f"xT{j}", name=f"xT{j}") for j in range(FT)]
            wq = [persist.tile([128, H], dt.bfloat16, tag=f"wq{j}", name=f"wq{j}") for j in range(FT)]
            wk = [persist.tile([128, H], dt.bfloat16, tag=f"wk{j}", name=f"wk{j}") for j in range(FT)]
            wv = [persist.tile([128, H], dt.bfloat16, tag=f"wv{j}", name=f"wv{j}") for j in range(FT)]
            wo = [persist.tile([128, H], dt.bfloat16, tag=f"wo{j}", name=f"wo{j}") for j in range(FT)]
            bq = [persist.tile([128, 1], dt.float32, tag=f"bq{j}", name=f"bq{j}") for j in range(FT)]
            bk = [persist.tile([128, 1], dt.float32, tag=f"bk{j}", name=f"bk{j}") for j in range(FT)]
            bv = persist.tile([1, H], dt.bfloat16, tag="bv", name="bv")
            bo = persist.tile([1, H], dt.bfloat16, tag="bo", name="bo")
            mb = persist.tile([128, KT_TILES], dt.float32, tag="mb", name="mb")
            ones = persist.tile([1, 128], dt.bfloat16, tag="ones", name="ones")

            for j in range(FT):
                nc.sync.dma_start(out=xT[j], in_=d_xT[j])
                nc.sync.dma_start(out=wq[j], in_=d_wq[j])
                nc.sync.dma_start(out=wk[j], in_=d_wk[j])
                nc.sync.dma_start(out=wv[j], in_=d_wv[j])
                nc.sync.dma_start(out=wo[j], in_=d_wo[j])
                nc.sync.dma_start(out=bq[j], in_=d_bq[j])
                nc.sync.dma_start(out=bk[j], in_=d_bk[j])
            nc.sync.dma_start(out=bv, in_=d_bv[:, :])
            nc.sync.dma_start(out=bo, in_=d_bo[:, :])
            nc.sync.dma_start(out=mb, in_=d_mb[:, :])
            nc.vector.memset(ones, 1.0)

            QT = [persist.tile([128, SQ], dt.bfloat16, tag=f"QT{f}", name=f"QT{f}") for f in range(FT)]
            KT = [persist.tile([128, S], dt.bfloat16, tag=f"KT{f}", name=f"KT{f}") for f in range(FT)]
            # per-head block = [64 value cols | 64 ones cols]; the ones columns
            # make attn@V emit softmax sums as a full 64-row PSUM block, so
            # normalization is a single DVE divide (no partition broadcast).
            Vt = [persist.tile([128, NH, 2 * DH], dt.bfloat16, tag=f"V{s}", name=f"V{s}")
                  for s in range(ST)]
            valsT = [persist.tile([128, SQ], dt.bfloat16, tag=f"vT{f}", name=f"vT{f}")
                     for f in range(FT)]

            # ---- QT / KT projections for feature tile 0 first (unblocks attn) ----
            def proj_qkt(f):
                for w in range(N_QW):
                    ps = pp.tile([128, QW], dt.float32, tag="pp", name="pp")
                    for j in range(FT):
                        nc.tensor.matmul(
                            ps, lhsT=wq[j][:, f * 128:(f + 1) * 128],
                            rhs=xT[j][:, w * QW:(w + 1) * QW],
                            start=(j == 0), stop=(j == FT - 1))
                    nc.vector.tensor_scalar_add(
                        QT[f][:, w * QW:(w + 1) * QW], ps, bq[f])
                for w in range(S // QW):
                    ps = pp.tile([128, QW], dt.float32, tag="pp", name="pp")
                    for j in range(FT):
                        nc.tensor.matmul(
                            ps, lhsT=wk[j][:, f * 128:(f + 1) * 128],
                            rhs=xT[j][:, w * QW:(w + 1) * QW],
                            start=(j == 0), stop=(j == FT - 1))
                    nc.vector.tensor_scalar_add(
                        KT[f][:, w * QW:(w + 1) * QW], ps, bk[f])

            proj_qkt(0)

            # ---- V projection (keys x values, all heads), ones column ----
            for s in range(ST):
                ps = pp.tile([128, H], dt.float32, tag="pp", name="pp")
                for j in range(FT):
                    nc.tensor.matmul(
                        ps, lhsT=xT[j][:, s * 128:(s + 1) * 128], rhs=wv[j],
                        start=(j == 0), stop=False)
                nc.tensor.matmul(ps, lhsT=ones, rhs=bv, start=False, stop=True)
                nc.vector.tensor_copy(
                    Vt[s][:, :, 0:DH],
                    ps.rearrange("p (h d) -> p h d", h=NH))
                nc.vector.memset(Vt[s][:, :, DH:2 * DH], 1.0)

            for f in range(1, FT):
                proj_qkt(f)

            # ---- attention: head pairs (features of tile hp = heads 2hp, 2hp+1) ----
            for hp in range(FT):
                for w in range(N_QW):
                    qs = slice(w * QW, (w + 1) * QW)
                    pts = []
                    for kt in range(KT_TILES):
                        lg = lgp.tile([128, 2 * QW], dt.float32, tag="lg", name="lg")
                        nc.tensor.matmul(
                            lg[:, 0:QW],
                            lhsT=KT[hp][0:64, kt * 128:(kt + 1) * 128],
                            rhs=QT[hp][0:64, qs],
                            start=True, stop=True, tile_position=(0, 0))
                        nc.tensor.matmul(
                            lg[:, QW:2 * QW],
                            lhsT=KT[hp][64:128, kt * 128:(kt + 1) * 128],
                            rhs=QT[hp][64:128, qs],
                            start=True, stop=True, tile_position=(64, 0))
                        pt = ptp.tile([128, 2 * QW], dt.bfloat16, tag="pt", name="pt")
                        nc.scalar.activation(
                            pt, lg, AF.Exp,
                            bias=mb[:, kt:kt + 1], scale=SCALE)
                        pts.append(pt)
                    att = [avp.tile([128, QW], dt.float32, tag="av", name="av") for _ in range(2)]
                    for i in range(2):
                        for kt in range(KT_TILES):
                            nc.tensor.matmul(
                                att[i],
                                lhsT=Vt[kt][:, 2 * hp + i, :],
                                rhs=pts[kt][:, i * QW:(i + 1) * QW],
                                start=(kt == 0), stop=(kt == KT_TILES - 1))
                    for i in range(2):
                        rc = work.tile([64, QW], dt.float32, tag="rc", name="rc")
                        nc.vector.reciprocal(rc, att[i][DH:2 * DH, :])
                        nc.vector.tensor_mul(
                            valsT[hp][i * 64:(i + 1) * 64, qs],
                            att[i][0:DH, :], rc)

            # ---- output projection ----
            for s in range(SOT):
                ps = pp.tile([128, H], dt.float32, tag="pp", name="pp")
                for f in range(FT):
                    nc.tensor.matmul(
                        ps, lhsT=valsT[f][:, s * 128:(s + 1) * 128], rhs=wo[f],
                        start=(f == 0), stop=False)
                nc.tensor.matmul(ps, lhsT=ones, rhs=bo, start=False, stop=True)
                ot = outp.tile([128, H], dt.float32, tag="ot", name="ot")
                nc.vector.tensor_copy(ot, ps)
                nc.sync.dma_start(out=d_out[s * 128:(s + 1) * 128, :], in_=ot)

    nc.compile()
    return nc


def _get_nc():
    if "nc" not in _CACHE:
        _CACHE["nc"] = _build_nc()
    return _CACHE["nc"]


def _bf16(a):
    return np.ascontiguousarray(a.astype(ml_dtypes.bfloat16))


def _prep_shared(qkv_w, qkv_b, o_w, o_b):
    """Host-side weight permutation: reference splits qkv per head into
    (q, v, k) chunks of 64 within each head's 192 rows."""
    idx = np.arange(NH)[:, None] * (3 * DH) + np.arange(DH)[None, :]
    q_idx = idx.ravel()            # head-major q rows
    v_idx = (idx + DH).ravel()
    k_idx = (idx + 2 * DH).ravel()

    def wT(rows):
        # [H_in, 512 features] -> [FT, 128, 512]
        return _bf16(qkv_w[rows].T.reshape(H, H)).reshape(FT, 128, H)

    shared = {
        "wq": wT(q_idx), "wk": wT(k_idx), "wv": wT(v_idx),
        "wo": _bf16(o_w.T).reshape(FT, 128, H),
        "bq": np.ascontiguousarray(qkv_b[q_idx].astype(np.float32)).reshape(FT, 128, 1),
        "bk": np.ascontiguousarray(qkv_b[k_idx].astype(np.float32)).reshape(FT, 128, 1),
        "bv": _bf16(qkv_b[v_idx]).reshape(1, H),
        "bo": _bf16(o_b).reshape(1, H),
    }
    return shared


def kernel(x, src_padding_mask, qkv_w, qkv_b, o_w, o_b):
    from concourse.bass_utils import run_bass_kernel_spmd

    x = np.asarray(x, dtype=np.float32)
    mask = np.asarray(src_padding_mask)
    qkv_w = np.asarray(qkv_w, dtype=np.float32)
    qkv_b = np.asarray(qkv_b, dtype=np.float32)
    o_w = np.asarray(o_w, dtype=np.float32)
    o_b = np.asarray(o_b, dtype=np.float32)

    nc = _get_nc()
    shared = _prep_shared(qkv_w, qkv_b, o_w, o_b)

    in_maps = []
    for c in range(N_CORES):
        b, half = c // 2, c % 2
        xr = np.roll(x[b], -SQ * half, axis=0)     # first SQ rows = this core's q
        mr = np.roll(mask[b], -SQ * half)
        mbias = np.where(mr, 0.0, NEG_BIAS).astype(np.float32)
        m = dict(shared)
        m["xT"] = _bf16(xr.T).reshape(FT, 128, S)
        m["mb"] = np.ascontiguousarray(mbias.reshape(KT_TILES, 128).T)
        in_maps.append(m)

    res = run_bass_kernel_spmd(nc, in_maps, core_ids=list(range(N_CORES)))

    out = np.empty((B, S, H), dtype=np.float32)
    for c in range(N_CORES):
        b, half = c // 2, c % 2
        out[b, half * SQ:(half + 1) * SQ] = res.results[c]["out"]
    return out


# revision 12
# speedup vs baseline: 1.0333x; 1.0256x over previous
"""Multi-head attention (B=4, S=2048, H=512, NH=8) on 8 trn2 NeuronCores.

Sharding: core c handles batch b = c//2, query half = c%2 (1024 queries),
full 2048-key attention for all 8 heads, plus its slice of the QKV/out
projections.  No cross-core reduction is needed: each core produces final
output rows [b, half*1024:(half+1)*1024, :].

Per-core layout strategy (everything transposed, features on partitions):
  xT     [512, 2048]  input transposed (rotated so first 1024 cols = queries)
  QT     [512, 1024]  q features x queries          (bf16)
  KT     [512, 2048]  k features x keys             (bf16)
  V      [2048, 8*65] keys x (per-head 64 vals + ones col)  (bf16)
  logitsT[k, q] tiles via row-tiled (2 heads concurrent) K=64 matmuls
  pT = exp(scale*logitsT + maskbias_k): mask bias is per-partition (k on
       partitions), no max subtraction (logits bounded ~ +-10 after scale)
  attn.V: V-block stationary (M=65: 64 vals + ones col -> softmax sums ride
       along as PSUM row 64), pT moving
  normalize: vals / sums via reciprocal + DMA partition-broadcast + DVE mul
  o-proj: valsT [f, s] slices are the stationary operand directly -> out [s, 512]
"""

import numpy as np
import ml_dtypes

B, S, H, NH, DH = 4, 2048, 512, 8, 64
SQ = S // 2          # queries per core
N_CORES = 8
SCALE = 1.0 / 8.0    # 1/sqrt(64)
NEG_BIAS = -1e9      # pre-scale additive mask bias for padded keys

KT_TILES = S // 128       # 16 key tiles of 128
QW = 512                  # query window (psum free size)
N_QW = SQ // QW           # 2
FT = H // 128             # 4 feature tiles
ST = S // 128             # 16 s-tiles for V
SOT = SQ // 128           # 8 s-tiles for output

_CACHE = {}


def _build_nc():
    import concourse.bass as bass
    import concourse.mybir as mybir
    import concourse.tile as tile
    from concourse import bacc

    dt = mybir.dt
    AF = mybir.ActivationFunctionType

    nc = bacc.Bacc("TRN2", target_bir_lowering=False)

    # ---- DRAM I/O (per-core) ----
    d_xT = nc.dram_tensor("xT", [FT, 128, S], dt.bfloat16, kind="ExternalInput")
    d_wq = nc.dram_tensor("wq", [FT, 128, H], dt.bfloat16, kind="ExternalInput")
    d_wk = nc.dram_tensor("wk", [FT, 128, H], dt.bfloat16, kind="ExternalInput")
    d_wv = nc.dram_tensor("wv", [FT, 128, H], dt.bfloat16, kind="ExternalInput")
    d_wo = nc.dram_tensor("wo", [FT, 128, H], dt.bfloat16, kind="ExternalInput")
    d_bq = nc.dram_tensor("bq", [FT, 128, 1], dt.float32, kind="ExternalInput")
    d_bk = nc.dram_tensor("bk", [FT, 128, 1], dt.float32, kind="ExternalInput")
    d_mb = nc.dram_tensor("mb", [128, KT_TILES], dt.float32, kind="ExternalInput")
    d_out = nc.dram_tensor("out", [SQ, H], dt.float32, kind="ExternalOutput")

    with tile.TileContext(nc) as tc:
        with (
            tc.tile_pool(name="persist", bufs=1) as persist,
            tc.tile_pool(name="pt", bufs=20) as ptp,
            tc.tile_pool(name="work", bufs=4) as work,
            tc.tile_pool(name="outp", bufs=4) as outp,
            tc.tile_pool(name="pp", bufs=2, space="PSUM") as pp,
            tc.tile_pool(name="lgp", bufs=2, space="PSUM") as lgp,
            tc.tile_pool(name="avp", bufs=2, space="PSUM") as avp,
        ):
            # ---- load persistent tensors ----
            xT = [persist.tile([128, S], dt.bfloat16, tag=f"xT{j}", name=f"xT{j}") for j in range(FT)]
            wq = [persist.tile([128, H], dt.bfloat16, tag=f"wq{j}", name=f"wq{j}") for j in range(FT)]
            wk = [persist.tile([128, H], dt.bfloat16, tag=f"wk{j}", name=f"wk{j}") for j in range(FT)]
            wv = [persist.tile([128, H], dt.bfloat16, tag=f"wv{j}", name=f"wv{j}") for j in range(FT)]
            wo = [persist.tile([128, H], dt.bfloat16, tag=f"wo{j}", name=f"wo{j}") for j in range(FT)]
            bq = [persist.tile([128, 1], dt.float32, tag=f"bq{j}", name=f"bq{j}") for j in range(FT)]
            bk = [persist.tile([128, 1], dt.float32, tag=f"bk{j}", name=f"bk{j}") for j in range(FT)]
            mb = persist.tile([128, KT_TILES], dt.float32, tag="mb", name="mb")

            for j in range(FT):
                nc.sync.dma_start(out=xT[j], in_=d_xT[j])
                nc.sync.dma_start(out=wq[j], in_=d_wq[j])
                nc.sync.dma_start(out=wk[j], in_=d_wk[j])
                nc.sync.dma_start(out=wv[j], in_=d_wv[j])
                nc.sync.dma_start(out=wo[j], in_=d_wo[j])
                nc.sync.dma_start(out=bq[j], in_=d_bq[j])
                nc.sync.dma_start(out=bk[j], in_=d_bk[j])
            nc.sync.dma_start(out=mb, in_=d_mb[:, :])

            # QT is stored zero-padded per head so the logits matmul runs with
            # K=128 (same tile mode as every other matmul -> no PE mode-switch
            # drains): QTp[f][0] has head 2f's features in rows 0-63 and zeros
            # in 64-127; QTp[f][1] is the mirror.  lhsT is then the full
            # two-head KT block; the zero rows null the other head's term.
            QTp = [[persist.tile([128, SQ], dt.bfloat16, tag=f"QT{f}_{i}",
                                 name=f"QT{f}_{i}") for i in range(2)]
                   for f in range(FT)]
            KT = [persist.tile([128, S], dt.bfloat16, tag=f"KT{f}", name=f"KT{f}") for f in range(FT)]
            # per-head block = [64 value cols | 64 ones cols]; the ones columns
            # make attn@V emit softmax sums as a full 64-row PSUM block, so
            # normalization is reciprocal + multiply on 64 lanes.
            Vt = [persist.tile([128, NH, 2 * DH], dt.bfloat16, tag=f"V{s}", name=f"V{s}")
                  for s in range(ST)]
            valsT = [persist.tile([128, SQ], dt.bfloat16, tag=f"vT{f}", name=f"vT{f}")
                     for f in range(FT)]

            for f in range(FT):
                nc.vector.memset(QTp[f][0][64:128, :], 0.0)
                nc.vector.memset(QTp[f][1][0:64, :], 0.0)

            def emit_qt_group(f, w):
                ps = pp.tile([128, QW], dt.float32, tag="pp", name="pp")
                for j in range(FT):
                    nc.tensor.matmul(
                        ps, lhsT=wq[j][:, f * 128:(f + 1) * 128],
                        rhs=xT[j][:, w * QW:(w + 1) * QW],
                        start=(j == 0), stop=(j == FT - 1))
                nc.vector.tensor_scalar_add(
                    QTp[f][0][0:64, w * QW:(w + 1) * QW], ps[0:64, :], bq[f][0:64])
                nc.vector.tensor_scalar_add(
                    QTp[f][1][64:128, w * QW:(w + 1) * QW], ps[64:128, :],
                    bq[f][64:128])

            def emit_kt_group(f, w):
                ps = pp.tile([128, QW], dt.float32, tag="pp", name="pp")
                for j in range(FT):
                    nc.tensor.matmul(
                        ps, lhsT=wk[j][:, f * 128:(f + 1) * 128],
                        rhs=xT[j][:, w * QW:(w + 1) * QW],
                        start=(j == 0), stop=(j == FT - 1))
                nc.vector.tensor_scalar_add(
                    KT[f][:, w * QW:(w + 1) * QW], ps, bk[f])

            def emit_v_group(s):
                ps = pp.tile([128, H], dt.float32, tag="pp", name="pp")
                for j in range(FT):
                    nc.tensor.matmul(
                        ps, lhsT=xT[j][:, s * 128:(s + 1) * 128], rhs=wv[j],
                        start=(j == 0), stop=(j == FT - 1))
                nc.vector.tensor_copy(
                    Vt[s][:, :, 0:DH],
                    ps.rearrange("p (h d) -> p h d", h=NH))
                nc.vector.memset(Vt[s][:, :, DH:2 * DH], 1.0)

            def emit_oproj(s):
                ps = pp.tile([128, H], dt.float32, tag="pp", name="pp")
                for f in range(FT):
                    nc.tensor.matmul(
                        ps, lhsT=valsT[f][:, s * 128:(s + 1) * 128], rhs=wo[f],
                        start=(f == 0), stop=(f == FT - 1))
                ot = outp.tile([128, H], dt.float32, tag="ot", name="ot")
                nc.vector.tensor_copy(ot, ps)
                nc.sync.dma_start(out=d_out[s * 128:(s + 1) * 128, :], in_=ot)

            # head: the minimum projections attention iteration 0 needs
            for w in range(N_QW):
                emit_qt_group(0, w)
            for w in range(S // QW):
                emit_kt_group(0, w)
            emit_v_group(0)
            emit_v_group(1)

            # remaining projection work, fed into PE gaps of the ACT-paced
            # attention stream: V first (needed by iteration 0's attn@V),
            # then QT/KT of feature tiles 1-3 (needed by iterations 1-3).
            fillers = [(emit_v_group, (s,)) for s in range(2, ST)]
            for f in range(1, FT):
                for w in range(N_QW):
                    fillers.append((emit_qt_group, (f, w)))
                for w in range(S // QW):
                    fillers.append((emit_kt_group, (f, w)))

            # ---- attention: ACT-paced pipeline, attn@V(kt-1) interleaved ----
            # hp-outer so feature tile f is first consumed at iteration 2f,
            # giving the filler stream time to produce it (emission order IS
            # dataflow order for Tile).
            oproj_fill = []
            for hp in range(FT):
                for w in range(N_QW):
                    it = hp * N_QW + w
                    qs = slice(w * QW, (w + 1) * QW)
                    pts = []
                    att = [avp.tile([128, QW], dt.float32, tag="av", name="av")
                           for _ in range(2)]

                    def attnv(kt):
                        for i in range(2):
                            nc.tensor.matmul(
                                att[i],
                                lhsT=Vt[kt][:, 2 * hp + i, :],
                                rhs=pts[kt][:, i * QW:(i + 1) * QW],
                                start=(kt == 0), stop=(kt == KT_TILES - 1))

                    for kt in range(KT_TILES):
                        lg = lgp.tile([128, 2 * QW], dt.float32, tag="lg", name="lg")
                        for i in range(2):
                            nc.tensor.matmul(
                                lg[:, i * QW:(i + 1) * QW],
                                lhsT=KT[hp][:, kt * 128:(kt + 1) * 128],
                                rhs=QTp[hp][i][:, qs],
                                start=True, stop=True)
                        pt = ptp.tile([128, 2 * QW], dt.bfloat16, tag="pt", name="pt")
                        nc.scalar.activation(
                            pt, lg, AF.Exp,
                            bias=mb[:, kt:kt + 1], scale=SCALE)
                        pts.append(pt)
                        if kt > 0:
                            attnv(kt - 1)
                        if it == 0 and kt < 14 and fillers:
                            fn, args = fillers.pop(0)
                            fn(*args)
                        elif 1 <= it <= 3 and kt % 3 == 0 and fillers:
                            fn, args = fillers.pop(0)
                            fn(*args)
                        elif it == 7 and kt % 4 == 0 and oproj_fill:
                            emit_oproj(oproj_fill.pop(0))
                    attnv(KT_TILES - 1)

                    for i in range(2):
                        rc = work.tile([64, QW], dt.float32, tag="rc", name="rc")
                        nc.vector.reciprocal(rc, att[i][DH:2 * DH, :])
                        nc.vector.tensor_mul(
                            valsT[hp][i * 64:(i + 1) * 64, qs],
                            att[i][0:DH, :], rc)
                    if hp == FT - 1 and w == 0:
                        # all heads done for query window 0 after (hp3, qw0):
                        # its output projection rides iteration 7 as filler
                        oproj_fill = list(range(0, QW // 128))

            # ---- remaining output projection (query window 1) ----
            for s in range(QW // 128, SOT):
                emit_oproj(s)
            for s in oproj_fill:
                emit_oproj(s)

    nc.compile()
    return nc


def _get_nc():
    if "nc" not in _CACHE:
        _CACHE["nc"] = _build_nc()
    return _CACHE["nc"]


def _bf16(a):
    return np.ascontiguousarray(a.astype(ml_dtypes.bfloat16))


def _prep_shared(qkv_w, qkv_b, o_w, o_b):
    """Host-side weight permutation: reference splits qkv per head into
    (q, v, k) chunks of 64 within each head's 192 rows."""
    idx = np.arange(NH)[:, None] * (3 * DH) + np.arange(DH)[None, :]
    q_idx = idx.ravel()            # head-major q rows
    v_idx = (idx + DH).ravel()
    k_idx = (idx + 2 * DH).ravel()

    def wT(rows):
        # [H_in, 512 features] -> [FT, 128, 512]
        return _bf16(qkv_w[rows].T.reshape(H, H)).reshape(FT, 128, H)

    shared = {
        "wq": wT(q_idx), "wk": wT(k_idx), "wv": wT(v_idx),
        "wo": _bf16(o_w.T).reshape(FT, 128, H),
        "bq": np.ascontiguousarray(qkv_b[q_idx].astype(np.float32)).reshape(FT, 128, 1),
        "bk": np.ascontiguousarray(qkv_b[k_idx].astype(np.float32)).reshape(FT, 128, 1),
    }
    # softmax rows sum to 1, so the value-projection bias contributes
    # qkv_b[v] @ o_w.T to every output row; fold it with o_b host-side.
    bias_row = (qkv_b[v_idx].astype(np.float64) @ o_w.T.astype(np.float64)
                + o_b.astype(np.float64)).astype(np.float32)
    return shared, bias_row


def kernel(x, src_padding_mask, qkv_w, qkv_b, o_w, o_b):
    from concourse.bass_utils import run_bass_kernel_spmd

    x = np.asarray(x, dtype=np.float32)
    mask = np.asarray(src_padding_mask)
    qkv_w = np.asarray(qkv_w, dtype=np.float32)
    qkv_b = np.asarray(qkv_b, dtype=np.float32)
    o_w = np.asarray(o_w, dtype=np.float32)
    o_b = np.asarray(o_b, dtype=np.float32)

    nc = _get_nc()
    shared, bias_row = _prep_shared(qkv_w, qkv_b, o_w, o_b)

    in_maps = []
    for c in range(N_CORES):
        b, half = c // 2, c % 2
        xr = np.roll(x[b], -SQ * half, axis=0)     # first SQ rows = this core's q
        mr = np.roll(mask[b], -SQ * half)
        mbias = np.where(mr, 0.0, NEG_BIAS).astype(np.float32)
        m = dict(shared)
        m["xT"] = _bf16(xr.T).reshape(FT, 128, S)
        m["mb"] = np.ascontiguousarray(mbias.reshape(KT_TILES, 128).T)
        in_maps.append(m)

    res = run_bass_kernel_spmd(nc, in_maps, core_ids=list(range(N_CORES)))

    out = np.empty((B, S, H), dtype=np.float32)
    for c in range(N_CORES):
        b, half = c // 2, c % 2
        out[b, half * SQ:(half + 1) * SQ] = res.results[c]["out"]
    out += bias_row
    return out


# revision 13
# speedup vs baseline: 1.0720x; 1.0374x over previous
"""Multi-head attention (B=4, S=2048, H=512, NH=8) on 8 trn2 NeuronCores.

Sharding: core c handles batch b = c//2, query half = c%2 (1024 queries),
full 2048-key attention for all 8 heads, plus its slice of the QKV/out
projections.  No cross-core reduction is needed: each core produces final
output rows [b, half*1024:(half+1)*1024, :].

Per-core layout strategy (everything transposed, features on partitions):
  xT     [512, 2048]  input transposed (rotated so first 1024 cols = queries)
  QT     [512, 1024]  q features x queries          (bf16)
  KT     [512, 2048]  k features x keys             (bf16)
  V      [2048, 8*65] keys x (per-head 64 vals + ones col)  (bf16)
  logitsT[k, q] tiles via row-tiled (2 heads concurrent) K=64 matmuls
  pT = exp(scale*logitsT + maskbias_k): mask bias is per-partition (k on
       partitions), no max subtraction (logits bounded ~ +-10 after scale)
  attn.V: V-block stationary (M=65: 64 vals + ones col -> softmax sums ride
       along as PSUM row 64), pT moving
  normalize: vals / sums via reciprocal + DMA partition-broadcast + DVE mul
  o-proj: valsT [f, s] slices are the stationary operand directly -> out [s, 512]
"""

import numpy as np
import ml_dtypes

B, S, H, NH, DH = 4, 2048, 512, 8, 64
SQ = S // 2          # queries per core
N_CORES = 8
SCALE = 1.0 / 8.0    # 1/sqrt(64)
NEG_BIAS = -1e9      # pre-scale additive mask bias for padded keys

KT_TILES = S // 128       # 16 key tiles of 128
QW = 512                  # query window (psum free size)
N_QW = SQ // QW           # 2
FT = H // 128             # 4 feature tiles
ST = S // 128             # 16 s-tiles for V
SOT = SQ // 128           # 8 s-tiles for output

_CACHE = {}


def _build_nc():
    import concourse.bass as bass
    import concourse.mybir as mybir
    import concourse.tile as tile
    from concourse import bacc

    dt = mybir.dt
    AF = mybir.ActivationFunctionType

    nc = bacc.Bacc("TRN2", target_bir_lowering=False)

    # ---- DRAM I/O (per-core) ----
    d_xT = nc.dram_tensor("xT", [FT, 128, S], dt.bfloat16, kind="ExternalInput")
    d_wq = nc.dram_tensor("wq", [FT, 128, H], dt.bfloat16, kind="ExternalInput")
    d_wk = nc.dram_tensor("wk", [FT, 128, H], dt.bfloat16, kind="ExternalInput")
    d_wv = nc.dram_tensor("wv", [FT, 128, H], dt.bfloat16, kind="ExternalInput")
    d_wo = nc.dram_tensor("wo", [FT, 128, H], dt.bfloat16, kind="ExternalInput")
    d_bq = nc.dram_tensor("bq", [FT, 128, 1], dt.float32, kind="ExternalInput")
    d_bk = nc.dram_tensor("bk", [FT, 128, 1], dt.float32, kind="ExternalInput")
    d_mb = nc.dram_tensor("mb", [128, KT_TILES], dt.float32, kind="ExternalInput")
    d_out = nc.dram_tensor("out", [SQ, H], dt.float32, kind="ExternalOutput")

    with tile.TileContext(nc) as tc:
        with (
            tc.tile_pool(name="persist", bufs=1) as persist,
            tc.tile_pool(name="pt", bufs=20) as ptp,
            tc.tile_pool(name="work", bufs=4) as work,
            tc.tile_pool(name="outp", bufs=4) as outp,
            tc.tile_pool(name="pp", bufs=2, space="PSUM") as pp,
            tc.tile_pool(name="lgp", bufs=2, space="PSUM") as lgp,
            tc.tile_pool(name="avp", bufs=2, space="PSUM") as avp,
        ):
            # ---- load persistent tensors ----
            xT = [persist.tile([128, S], dt.bfloat16, tag=f"xT{j}", name=f"xT{j}") for j in range(FT)]
            wq = [persist.tile([128, H], dt.bfloat16, tag=f"wq{j}", name=f"wq{j}") for j in range(FT)]
            wk = [persist.tile([128, H], dt.bfloat16, tag=f"wk{j}", name=f"wk{j}") for j in range(FT)]
            wv = [persist.tile([128, H], dt.bfloat16, tag=f"wv{j}", name=f"wv{j}") for j in range(FT)]
            wo = [persist.tile([128, H], dt.bfloat16, tag=f"wo{j}", name=f"wo{j}") for j in range(FT)]
            bq = [persist.tile([128, 1], dt.float32, tag=f"bq{j}", name=f"bq{j}") for j in range(FT)]
            bk = [persist.tile([128, 1], dt.float32, tag=f"bk{j}", name=f"bk{j}") for j in range(FT)]
            mb = persist.tile([128, KT_TILES], dt.float32, tag="mb", name="mb")

            for j in range(FT):
                nc.sync.dma_start(out=xT[j], in_=d_xT[j])
                nc.sync.dma_start(out=wq[j], in_=d_wq[j])
                nc.sync.dma_start(out=wk[j], in_=d_wk[j])
                nc.sync.dma_start(out=wv[j], in_=d_wv[j])
                nc.sync.dma_start(out=wo[j], in_=d_wo[j])
                nc.sync.dma_start(out=bq[j], in_=d_bq[j])
                nc.sync.dma_start(out=bk[j], in_=d_bk[j])
            nc.sync.dma_start(out=mb, in_=d_mb[:, :])

            QT = [persist.tile([128, SQ], dt.bfloat16, tag=f"QT{f}",
                               name=f"QT{f}") for f in range(FT)]
            KT = [persist.tile([128, S], dt.bfloat16, tag=f"KT{f}", name=f"KT{f}") for f in range(FT)]
            # per-head block = [64 value cols | 64 ones cols]; the ones columns
            # make attn@V emit softmax sums as a full 64-row PSUM block, so
            # normalization is reciprocal + multiply on 64 lanes.
            Vt = [persist.tile([128, NH, 2 * DH], dt.bfloat16, tag=f"V{s}", name=f"V{s}")
                  for s in range(ST)]
            valsT = [persist.tile([128, SQ], dt.bfloat16, tag=f"vT{f}", name=f"vT{f}")
                     for f in range(FT)]

            def emit_qt_group(f, w):
                ps = pp.tile([128, QW], dt.float32, tag="pp", name="pp")
                for j in range(FT):
                    nc.tensor.matmul(
                        ps, lhsT=wq[j][:, f * 128:(f + 1) * 128],
                        rhs=xT[j][:, w * QW:(w + 1) * QW],
                        start=(j == 0), stop=(j == FT - 1))
                nc.vector.tensor_scalar_add(
                    QT[f][:, w * QW:(w + 1) * QW], ps, bq[f])

            def emit_kt_group(f, w):
                ps = pp.tile([128, QW], dt.float32, tag="pp", name="pp")
                for j in range(FT):
                    nc.tensor.matmul(
                        ps, lhsT=wk[j][:, f * 128:(f + 1) * 128],
                        rhs=xT[j][:, w * QW:(w + 1) * QW],
                        start=(j == 0), stop=(j == FT - 1))
                nc.vector.tensor_scalar_add(
                    KT[f][:, w * QW:(w + 1) * QW], ps, bk[f])

            def emit_v_group(s):
                ps = pp.tile([128, H], dt.float32, tag="pp", name="pp")
                for j in range(FT):
                    nc.tensor.matmul(
                        ps, lhsT=xT[j][:, s * 128:(s + 1) * 128], rhs=wv[j],
                        start=(j == 0), stop=(j == FT - 1))
                nc.vector.tensor_copy(
                    Vt[s][:, :, 0:DH],
                    ps.rearrange("p (h d) -> p h d", h=NH))
                nc.vector.memset(Vt[s][:, :, DH:2 * DH], 1.0)

            def emit_oproj(s):
                ps = pp.tile([128, H], dt.float32, tag="pp", name="pp")
                for f in range(FT):
                    nc.tensor.matmul(
                        ps, lhsT=valsT[f][:, s * 128:(s + 1) * 128], rhs=wo[f],
                        start=(f == 0), stop=(f == FT - 1))
                ot = outp.tile([128, H], dt.float32, tag="ot", name="ot")
                nc.vector.tensor_copy(ot, ps)
                nc.sync.dma_start(out=d_out[s * 128:(s + 1) * 128, :], in_=ot)

            # head: the minimum projections attention iteration 0 needs
            for w in range(N_QW):
                emit_qt_group(0, w)
            for w in range(S // QW):
                emit_kt_group(0, w)
            emit_v_group(0)
            emit_v_group(1)

            # remaining projection work, fed into PE gaps of the ACT-paced
            # attention stream: V first (needed by iteration 0's attn@V),
            # then QT/KT of feature tiles 1-3 (needed by iterations 1-3).
            fillers = [(emit_v_group, (s,)) for s in range(2, ST)]
            for f in range(1, FT):
                for w in range(N_QW):
                    fillers.append((emit_qt_group, (f, w)))
                for w in range(S // QW):
                    fillers.append((emit_kt_group, (f, w)))

            # ---- attention: ACT-paced pipeline, attn@V(kt-1) interleaved ----
            # hp-outer so feature tile f is first consumed at iteration 2f,
            # giving the filler stream time to produce it (emission order IS
            # dataflow order for Tile).
            oproj_fill = []
            for hp in range(FT):
                for w in range(N_QW):
                    it = hp * N_QW + w
                    qs = slice(w * QW, (w + 1) * QW)
                    pts = []
                    att = [avp.tile([128, QW], dt.float32, tag="av", name="av")
                           for _ in range(2)]

                    def attnv(kt):
                        for i in range(2):
                            nc.tensor.matmul(
                                att[i],
                                lhsT=Vt[kt][:, 2 * hp + i, :],
                                rhs=pts[kt][:, i * QW:(i + 1) * QW],
                                start=(kt == 0), stop=(kt == KT_TILES - 1))

                    for kt in range(KT_TILES):
                        lg = lgp.tile([128, 2 * QW], dt.float32, tag="lg", name="lg")
                        for i in range(2):
                            nc.tensor.matmul(
                                lg[:, i * QW:(i + 1) * QW],
                                lhsT=KT[hp][i * 64:(i + 1) * 64,
                                            kt * 128:(kt + 1) * 128],
                                rhs=QT[hp][i * 64:(i + 1) * 64, qs],
                                start=True, stop=True,
                                tile_position=(i * 64, 0))
                        pt = ptp.tile([128, 2 * QW], dt.bfloat16, tag="pt", name="pt")
                        nc.scalar.activation(
                            pt, lg, AF.Exp,
                            bias=mb[:, kt:kt + 1], scale=SCALE)
                        pts.append(pt)
                        if kt > 0:
                            attnv(kt - 1)
                        if it == 0 and kt < 14 and fillers:
                            fn, args = fillers.pop(0)
                            fn(*args)
                        elif 1 <= it <= 3 and kt % 3 == 0 and fillers:
                            fn, args = fillers.pop(0)
                            fn(*args)
                        elif it == 7 and kt % 4 == 0 and oproj_fill:
                            emit_oproj(oproj_fill.pop(0))
                    attnv(KT_TILES - 1)

                    for i in range(2):
                        rc = work.tile([64, QW], dt.float32, tag="rc", name="rc")
                        nc.vector.reciprocal(rc, att[i][DH:2 * DH, :])
                        nc.vector.tensor_mul(
                            valsT[hp][i * 64:(i + 1) * 64, qs],
                            att[i][0:DH, :], rc)
                    if hp == FT - 1 and w == 0:
                        # all heads done for query window 0 after (hp3, qw0):
                        # its output projection rides iteration 7 as filler
                        oproj_fill = list(range(0, QW // 128))

            # ---- remaining output projection (query window 1) ----
            for s in range(QW // 128, SOT):
                emit_oproj(s)
            for s in oproj_fill:
                emit_oproj(s)

    nc.compile()
    return nc


def _get_nc():
    if "nc" not in _CACHE:
        _CACHE["nc"] = _build_nc()
    return _CACHE["nc"]


def _bf16(a):
    return np.ascontiguousarray(a.astype(ml_dtypes.bfloat16))


def _prep_shared(qkv_w, qkv_b, o_w, o_b):
    """Host-side weight permutation: reference splits qkv per head into
    (q, v, k) chunks of 64 within each head's 192 rows."""
    idx = np.arange(NH)[:, None] * (3 * DH) + np.arange(DH)[None, :]
    q_idx = idx.ravel()            # head-major q rows
    v_idx = (idx + DH).ravel()
    k_idx = (idx + 2 * DH).ravel()

    def wT(rows):
        # [H_in, 512 features] -> [FT, 128, 512]
        return _bf16(qkv_w[rows].T.reshape(H, H)).reshape(FT, 128, H)

    shared = {
        "wq": wT(q_idx), "wk": wT(k_idx), "wv": wT(v_idx),
        "wo": _bf16(o_w.T).reshape(FT, 128, H),
        "bq": np.ascontiguousarray(qkv_b[q_idx].astype(np.float32)).reshape(FT, 128, 1),
        "bk": np.ascontiguousarray(qkv_b[k_idx].astype(np.float32)).reshape(FT, 128, 1),
    }
    # softmax rows sum to 1, so the value-projection bias contributes
    # qkv_b[v] @ o_w.T to every output row; fold it with o_b host-side.
    bias_row = (qkv_b[v_idx].astype(np.float64) @ o_w.T.astype(np.float64)
                + o_b.astype(np.float64)).astype(np.float32)
    return shared, bias_row


def kernel(x, src_padding_mask, qkv_w, qkv_b, o_w, o_b):
    from concourse.bass_utils import run_bass_kernel_spmd

    x = np.asarray(x, dtype=np.float32)
    mask = np.asarray(src_padding_mask)
    qkv_w = np.asarray(qkv_w, dtype=np.float32)
    qkv_b = np.asarray(qkv_b, dtype=np.float32)
    o_w = np.asarray(o_w, dtype=np.float32)
    o_b = np.asarray(o_b, dtype=np.float32)

    nc = _get_nc()
    shared, bias_row = _prep_shared(qkv_w, qkv_b, o_w, o_b)

    in_maps = []
    for c in range(N_CORES):
        b, half = c // 2, c % 2
        xr = np.roll(x[b], -SQ * half, axis=0)     # first SQ rows = this core's q
        mr = np.roll(mask[b], -SQ * half)
        mbias = np.where(mr, 0.0, NEG_BIAS).astype(np.float32)
        m = dict(shared)
        m["xT"] = _bf16(xr.T).reshape(FT, 128, S)
        m["mb"] = np.ascontiguousarray(mbias.reshape(KT_TILES, 128).T)
        in_maps.append(m)

    res = run_bass_kernel_spmd(nc, in_maps, core_ids=list(range(N_CORES)))

    out = np.empty((B, S, H), dtype=np.float32)
    for c in range(N_CORES):
        b, half = c // 2, c % 2
        out[b, half * SQ:(half + 1) * SQ] = res.results[c]["out"]
    out += bias_row
    return out
